# revision 14
# baseline (speedup 1.0000x reference)
"""GAT layer (2 blocks, concat) + per-graph sum pooling on 8 Trainium2 cores.

Strategy (self-contained; shapes hardcoded for the target problem):
  - Pad nodes 10000 -> 10240 = 8 cores x 10 tiles x 128. Core c owns dst
    nodes [c*1280, (c+1)*1280). Edges are bucketed by dst tile on the host.
  - Each core redundantly computes the full projected-feature table
    [feat(128) | ones | a_nb] (bf16, 256-wide rows = 512B, dma_gather's
    granularity) for all nodes via one fused matmul per 128-node tile
    (host pre-transposes x tiles; wl/wr fold into extra weight columns).
    a_self for own nodes stays in SBUF; x@Wres^T for own nodes in DRAM.
  - Node space is permuted per core (own nodes first) so all addressing is
    compile-time; edge index inputs are remapped accordingly.
  - Phase 2 per dst tile: ONE dma_gather (gpsimd ucode) fetches all J*128
    incident edges' table rows; scores p = exp(leakyrelu(a_self[dst] +
    a_nb[src])) are computed batched. a_self[dst] per edge comes from a
    PE broadcast of the host-provided dst-local row (ones x row outer
    product), an is_equal against the partition index (-> onehot^T), and
    a tiny matmul against the SBUF-resident a_self column. Per 128-edge
    subtile a bf16 selection matrix P_T[e,d] = (dst_local[e]==d)*p[e]
    (one DVE op) PSUM-accumulates P_T.T @ [feat|ones]: segment-softmax
    numerator and denominator fall out of one bf16 matmul chain
    (max-subtraction dropped: scores bounded ~14, exp safe in fp32).
  - h = relu(accum/denom + x@Wres^T); per-graph pooling is a one-hot
    matmul into a persistent PSUM tile; host sums the 8 partial phis.
"""
import numpy as np

import concourse.bass as bass
import concourse.tile as tile
from concourse import bacc, mybir
from concourse.bass_utils import run_bass_kernel_spmd

P = 128
D = 128
N_NODES = 10000
N_GRAPHS = 64
N_BLOCKS = 2
CORES = 8
NT = 80                 # node tiles (padded): NT*P = 10240
TPC = NT // CORES       # dst tiles owned per core
TW = 256                # bf16 table row: feat(128) | ones | a_nb | pad(126)
NEG_SLOPE = 0.01
CHUNK = 4               # subtiles per O^T broadcast chunk (<= 512 psum cols)

F32 = mybir.dt.float32
BF16 = mybir.dt.bfloat16
I16 = mybir.dt.int16

LAST_RESULTS = None     # stash for test harnesses
LAST_NC = None
LAST_IN_MAPS = None


def build_program(nt, tpc, J, n_graphs):
    """Build the SPMD Bass program (same program on all cores; per-core
    behavior comes only from input data)."""
    from contextlib import ExitStack

    nc = bacc.Bacc(
        "TRN2", target_bir_lowering=False, debug=False,
        enable_asserts=False, num_devices=1,
    )
    npad = nt * P
    own = tpc * P

    # ---- DRAM I/O ----
    xt = nc.dram_tensor("xt", [nt, P, P], F32, kind="ExternalInput")
    rhs_big = nc.dram_tensor("rhs_big", [N_BLOCKS, P, 259], F32, kind="ExternalInput")
    bias_big = nc.dram_tensor("bias_big", [N_BLOCKS, P, 259], F32, kind="ExternalInput")
    gidx = nc.dram_tensor("gidx", [tpc, P, 8 * J], I16, kind="ExternalInput")
    dstlocT = nc.dram_tensor("dstlocT", [tpc, 1, J * P], BF16, kind="ExternalInput")
    dstloc = nc.dram_tensor("dstloc", [tpc, P, J], F32, kind="ExternalInput")
    gidloc = nc.dram_tensor("gidloc", [tpc, P, 1], F32, kind="ExternalInput")
    iota_d = nc.dram_tensor("iota_d", [P, P], F32, kind="ExternalInput")
    iota_g = nc.dram_tensor("iota_g", [P, n_graphs], F32, kind="ExternalInput")
    iota_c = nc.dram_tensor("iota_c", [P, 1], F32, kind="ExternalInput")
    ones_r = nc.dram_tensor("ones_r", [1, P], BF16, kind="ExternalInput")
    ztile = nc.dram_tensor("ztile", [P, TW], BF16, kind="ExternalInput")

    h_own = nc.dram_tensor("h_own", [own, N_BLOCKS * D], F32, kind="ExternalOutput")
    phis_part = nc.dram_tensor("phis_part", [n_graphs, N_BLOCKS * D], F32,
                               kind="ExternalOutput")

    tables = [nc.dram_tensor(f"table{b}", [npad + 1, TW], BF16)
              for b in range(N_BLOCKS)]
    xress = [nc.dram_tensor(f"xres{b}", [own, D], F32) for b in range(N_BLOCKS)]

    with tile.TileContext(nc) as tc, ExitStack() as ctx:
        cpool = ctx.enter_context(tc.tile_pool(name="consts", bufs=1))
        p1 = ctx.enter_context(tc.tile_pool(name="p1", bufs=3))
        p1ps = ctx.enter_context(tc.tile_pool(name="p1ps", bufs=2, space="PSUM"))
        idxp = ctx.enter_context(tc.tile_pool(name="idxp", bufs=2))
        gp = ctx.enter_context(tc.tile_pool(name="gp", bufs=2))
        sm = ctx.enter_context(tc.tile_pool(name="sm", bufs=2))
        ptp = ctx.enter_context(tc.tile_pool(name="ptp", bufs=4))
        otp = ctx.enter_context(tc.tile_pool(name="otp", bufs=3))
        repps = ctx.enter_context(tc.tile_pool(name="repps", bufs=2, space="PSUM"))
        asps = ctx.enter_context(tc.tile_pool(name="asps", bufs=1, space="PSUM"))
        accps = ctx.enter_context(tc.tile_pool(name="accps", bufs=2, space="PSUM"))
        phips = ctx.enter_context(tc.tile_pool(name="phips", bufs=1, space="PSUM"))

        # ---- constants (host-provided; keeps gpsimd mlp-library-only) ----
        c_iota_d = cpool.tile([P, P], F32)
        nc.sync.dma_start(c_iota_d[:], iota_d[:])
        c_iota_g = cpool.tile([P, n_graphs], F32)
        nc.sync.dma_start(c_iota_g[:], iota_g[:])
        c_iota_c = cpool.tile([P, 1], F32)
        nc.sync.dma_start(c_iota_c[:], iota_c[:])
        c_ones = cpool.tile([1, P], BF16)
        nc.sync.dma_start(c_ones[:], ones_r[:])
        c_zt = cpool.tile([P, TW], BF16)
        nc.sync.dma_start(c_zt[:], ztile[:])
        rhs_sb, bias_sb, aself_sb = [], [], []
        for b in range(N_BLOCKS):
            r = cpool.tile([P, 259], F32, tag=f"rhs{b}")
            nc.sync.dma_start(r[:], rhs_big[b])
            rhs_sb.append(r)
            bb = cpool.tile([P, 259], F32, tag=f"bias{b}")
            nc.sync.dma_start(bb[:], bias_big[b])
            bias_sb.append(bb)
            asb = cpool.tile([P, tpc], F32, tag=f"aself{b}")
            aself_sb.append(asb)

        for b in range(N_BLOCKS):
            table, xres = tables[b], xress[b]
            # zero the whole table (incl. pad row + junk cols) first
            for k in range(0, npad + 1, P):
                rem = min(P, npad + 1 - k)
                nc.sync.dma_start(table[k:k + rem, :], c_zt[0:rem, :])

            # ---- phase 1: fused node projections ----
            # psum cols: 0:128 feat | 128 ones | 129 a_nb | 130 a_self | 131:259 xres
            for t in range(nt):
                xT = p1.tile([P, P], F32, tag="xT")
                nc.sync.dma_start(xT[:], xt[t])
                ps = p1ps.tile([P, 259], F32, tag="p1psum")
                nc.tensor.matmul(ps[:], lhsT=xT[:], rhs=rhs_sb[b][:],
                                 start=True, stop=True)
                osb = p1.tile([P, 259], F32, tag="osb")
                nc.vector.tensor_add(osb[:], ps[:], bias_sb[b][:])
                tbf = p1.tile([P, 130], BF16, tag="tbf")
                nc.scalar.copy(tbf[:], osb[:, 0:130])
                nc.sync.dma_start(table[t * P:(t + 1) * P, 0:130], tbf[:])
                if t < tpc:
                    nc.vector.tensor_copy(aself_sb[b][:, t:t + 1], osb[:, 130:131])
                    nc.sync.dma_start(xres[t * P:(t + 1) * P, :], osb[:, 131:259])

            # ---- phase 2: per-dst-tile edge aggregation ----
            php = phips.tile([n_graphs, D], F32, tag="php")
            for tt in range(tpc):
                t_gi = idxp.tile([P, 8 * J], I16, tag="tgi")
                nc.sync.dma_start(t_gi[:], gidx[tt])
                t_dlT = idxp.tile([1, J * P], BF16, tag="tdlT")
                nc.sync.dma_start(t_dlT[:], dstlocT[tt])
                t_dloc = idxp.tile([P, J], F32, tag="tdloc")
                nc.sync.dma_start(t_dloc[:], dstloc[tt])
                t_gid = idxp.tile([P, 1], F32, tag="tgid")
                nc.sync.dma_start(t_gid[:], gidloc[tt])

                G = gp.tile([P, J * TW], BF16, tag="G")
                nc.gpsimd.dma_gather(
                    G[:].rearrange("p (j w) -> p j w", j=J),
                    table[:],
                    t_gi[:],
                    J * P,
                    J * P,
                    TW,
                    single_packet=False,
                )

                # a_self[dst] per edge: PE row-broadcast -> onehot^T -> matmul
                ps_as = asps.tile([P, J], F32, tag="ps_as")
                for j0 in range(0, J, CHUNK):
                    cw = min(CHUNK, J - j0)
                    ps_rep = repps.tile([P, CHUNK * P], F32, tag="ps_rep")
                    nc.tensor.matmul(
                        ps_rep[:, 0:cw * P], lhsT=c_ones[:],
                        rhs=t_dlT[:, j0 * P:(j0 + cw) * P],
                        start=True, stop=True)
                    for j in range(j0, j0 + cw):
                        oT = otp.tile([P, P], F32, tag="oT")
                        nc.vector.tensor_scalar(
                            out=oT[:], in0=ps_rep[:, (j - j0) * P:(j - j0 + 1) * P],
                            scalar1=c_iota_c[:, 0:1], scalar2=None,
                            op0=mybir.AluOpType.is_equal)
                        nc.tensor.matmul(
                            ps_as[:, j:j + 1], lhsT=oT[:],
                            rhs=aself_sb[b][:, tt:tt + 1],
                            start=True, stop=True)

                # batched scores: p = exp(leakyrelu(a_self + a_nb))
                t_s = sm.tile([P, J], F32, tag="ts")
                nc.vector.tensor_add(
                    t_s[:],
                    G[:].rearrange("p (j w) -> p j w", j=J)[:, :, 129],
                    ps_as[:])
                t_u = sm.tile([P, J], F32, tag="tu")
                nc.vector.scalar_tensor_tensor(
                    out=t_u[:], in0=t_s[:], scalar=NEG_SLOPE, in1=t_s[:],
                    op0=mybir.AluOpType.mult, op1=mybir.AluOpType.max)
                t_p = sm.tile([P, J], F32, tag="tp")
                nc.scalar.activation(t_p[:], t_u[:], mybir.ActivationFunctionType.Exp)

                acc = accps.tile([P, D + 1], F32, tag="acc")
                for j in range(J):
                    PT = ptp.tile([P, P], BF16, tag="PT")
                    nc.vector.tensor_scalar(
                        out=PT[:], in0=c_iota_d[:],
                        scalar1=t_dloc[:, j:j + 1], scalar2=t_p[:, j:j + 1],
                        op0=mybir.AluOpType.is_equal, op1=mybir.AluOpType.mult)
                    nc.tensor.matmul(acc[:], lhsT=PT[:],
                                     rhs=G[:, j * TW:j * TW + D + 1],
                                     start=(j == 0), stop=(j == J - 1))

                # h = relu(accum/denom + xres)
                dmax = sm.tile([P, 1], F32, tag="dmax")
                nc.vector.tensor_scalar_max(dmax[:], acc[:, D:D + 1], 1e-30)
                rec = sm.tile([P, 1], F32, tag="rec")
                nc.vector.reciprocal(rec[:], dmax[:])
                xr = sm.tile([P, D], F32, tag="xr")
                nc.sync.dma_start(xr[:], xres[tt * P:(tt + 1) * P, :])
                hpre = sm.tile([P, D], F32, tag="hpre")
                nc.vector.scalar_tensor_tensor(
                    out=hpre[:], in0=acc[:, 0:D], scalar=rec[:, 0:1], in1=xr[:],
                    op0=mybir.AluOpType.mult, op1=mybir.AluOpType.add)
                ht = sm.tile([P, D], F32, tag="ht")
                nc.scalar.activation(ht[:], hpre[:], mybir.ActivationFunctionType.Relu)
                nc.sync.dma_start(
                    h_own[tt * P:(tt + 1) * P, b * D:(b + 1) * D], ht[:])

                # per-graph pooling: phis += onehot(gid).T @ h
                oh = sm.tile([P, n_graphs], F32, tag="oh")
                nc.vector.tensor_scalar(
                    out=oh[:], in0=c_iota_g[:], scalar1=t_gid[:, 0:1], scalar2=None,
                    op0=mybir.AluOpType.is_equal)
                nc.tensor.matmul(php[:], lhsT=oh[:], rhs=ht[:],
                                 start=(tt == 0), stop=(tt == tpc - 1))
                if tt == tpc - 1:
                    phsb = sm.tile([n_graphs, D], F32, tag="phsb")
                    nc.vector.tensor_copy(phsb[:], php[:])
                    nc.sync.dma_start(phis_part[:, b * D:(b + 1) * D], phsb[:])

    nc.compile()
    return nc


def host_prep(x, edge_src, edge_dst, graph_ids, Wfc, bfc, wl, wr, Wres,
              cores=CORES, nt=NT, ng=N_GRAPHS):
    """Build per-core input maps + the shared J (max subtiles per dst tile)."""
    tpc = nt // cores
    npad = nt * P
    own = tpc * P
    x = np.asarray(x, dtype=np.float32)
    edge_src = np.asarray(edge_src, dtype=np.int64)
    edge_dst = np.asarray(edge_dst, dtype=np.int64)
    graph_ids = np.asarray(graph_ids)
    Wfc = np.asarray(Wfc, dtype=np.float32)
    bfc = np.asarray(bfc, dtype=np.float32)
    wl = np.asarray(wl, dtype=np.float32)
    wr = np.asarray(wr, dtype=np.float32)
    Wres = np.asarray(Wres, dtype=np.float32)

    n = x.shape[0]
    x_pad = np.zeros((npad, D), dtype=np.float32)
    x_pad[:n] = x

    # fused weights: cols 0:128 Wfc^T | 128 zeros(->ones via bias) | 129 v_r |
    # 130 v_l | 131:259 Wres^T
    rhs_big = np.zeros((N_BLOCKS, P, 259), dtype=np.float32)
    bias_big = np.zeros((N_BLOCKS, P, 259), dtype=np.float32)
    for b in range(N_BLOCKS):
        rhs_big[b, :, 0:D] = Wfc[b].T
        rhs_big[b, :, 129] = Wfc[b].T @ wr[b]
        rhs_big[b, :, 130] = Wfc[b].T @ wl[b]
        rhs_big[b, :, 131:259] = Wres[b].T
        row = np.zeros(259, dtype=np.float32)
        row[0:D] = bfc[b]
        row[128] = 1.0
        row[129] = float(bfc[b] @ wr[b])
        row[130] = float(bfc[b] @ wl[b])
        bias_big[b, :, :] = row[None, :]

    import ml_dtypes
    consts = {
        "iota_d": np.tile(np.arange(P, dtype=np.float32)[None, :], (P, 1)),
        "iota_g": np.tile(np.arange(ng, dtype=np.float32)[None, :], (P, 1)),
        "iota_c": np.arange(P, dtype=np.float32)[:, None],
        "ones_r": np.ones((1, P), dtype=ml_dtypes.bfloat16),
        "ztile": np.zeros((P, TW), dtype=ml_dtypes.bfloat16),
    }

    # bucket edges by global dst tile
    order = np.argsort(edge_dst, kind="stable")
    sdst = edge_dst[order]
    ssrc = edge_src[order]
    tile_of = sdst // P
    counts = np.bincount(tile_of, minlength=nt)
    J = int(np.ceil(counts.max() / P))
    starts = np.zeros(nt + 1, dtype=np.int64)
    np.cumsum(counts, out=starts[1:])

    in_maps = []
    for c in range(cores):
        lo, hi = c * own, (c + 1) * own
        perm = np.concatenate([
            np.arange(lo, hi), np.arange(0, lo), np.arange(hi, npad)])
        pos = np.empty(npad, dtype=np.int64)
        pos[perm] = np.arange(npad)

        xtc = np.ascontiguousarray(
            x_pad[perm].reshape(nt, P, D).transpose(0, 2, 1))

        gidx_a = np.zeros((tpc, P, 8 * J), dtype=np.int16)
        dlT_a = np.full((tpc, 1, J * P), -1.0, dtype=ml_dtypes.bfloat16)
        dloc_a = np.full((tpc, P, J), -1.0, dtype=np.float32)
        for tt in range(tpc):
            g = c * tpc + tt
            s, e = starts[g], starts[g + 1]
            cnt = e - s
            flat_src = np.full(J * P, npad, dtype=np.int16)
            flat_src[:cnt] = pos[ssrc[s:e]].astype(np.int16)
            flat_dl = np.full(J * P, -1.0, dtype=np.float32)
            flat_dl[:cnt] = (sdst[s:e] - g * P).astype(np.float32)
            # wrapped-16 idx layout, replicated for the 8 gpsimd cores
            base = flat_src.reshape(8 * J, 16).T        # [16, 8J]
            gidx_a[tt] = np.tile(base, (8, 1))
            dlT_a[tt, 0] = flat_dl.astype(ml_dtypes.bfloat16)
            dloc_a[tt] = flat_dl.reshape(J, P).T

        gid_a = np.full((tpc, P, 1), -1.0, dtype=np.float32)
        own_nodes = np.arange(lo, hi)
        valid = own_nodes < n
        gv = np.full(own, -1.0, dtype=np.float32)
        gv[valid] = graph_ids[own_nodes[valid]].astype(np.float32)
        gid_a[:, :, 0] = gv.reshape(tpc, P)

        m = {"xt": xtc, "rhs_big": rhs_big, "bias_big": bias_big,
             "gidx": gidx_a, "dstlocT": dlT_a, "dstloc": dloc_a,
             "gidloc": gid_a}
        m.update(consts)
        in_maps.append(m)
    return in_maps, J


def kernel(x, edge_src, edge_dst, graph_ids, Wfc, bfc, wl, wr, Wres):
    global LAST_RESULTS, LAST_NC, LAST_IN_MAPS
    in_maps, J = host_prep(x, edge_src, edge_dst, graph_ids, Wfc, bfc, wl,
                           wr, Wres)
    nc = build_program(NT, TPC, J, N_GRAPHS)
    res = run_bass_kernel_spmd(nc, in_maps, list(range(CORES)))
    LAST_RESULTS = res
    LAST_NC, LAST_IN_MAPS = nc, in_maps

    n = np.asarray(x).shape[0]
    h = np.concatenate([r["h_own"] for r in res.results], axis=0)[:n]
    phis = np.sum([r["phis_part"] for r in res.results], axis=0,
                  dtype=np.float64).astype(np.float32)
    return h.astype(np.float32), phis


# revision 17
# speedup vs baseline: 24.2312x; 24.2312x over previous
"""GAT layer (2 blocks, concat) + per-graph sum pooling on 8 Trainium2 cores.

Strategy (self-contained; shapes hardcoded for the target problem):
  - Pad nodes 10000 -> 10240 = 8 cores x 10 tiles x 128. Core c owns dst
    nodes [c*1280, (c+1)*1280). Edges are bucketed by dst tile on the host.
  - Each core redundantly computes the full projected-feature table
    [feat(128) | ones | a_nb] (bf16, 256-wide rows = 512B, dma_gather's
    granularity) for all nodes via one fused matmul per 128-node tile
    (host pre-transposes x tiles; wl/wr fold into extra weight columns).
    a_self for own nodes stays in SBUF; x@Wres^T for own nodes in DRAM.
  - Node space is permuted per core (own nodes first) so all addressing is
    compile-time; edge index inputs are remapped accordingly.
  - Phase 2 per dst tile: ONE dma_gather (gpsimd ucode) fetches all J*128
    incident edges' table rows; scores p = exp(leakyrelu(a_self[dst] +
    a_nb[src])) are computed batched. a_self[dst] per edge comes from a
    PE broadcast of the host-provided dst-local row (ones x row outer
    product), an is_equal against the partition index (-> onehot^T), and
    a tiny matmul against the SBUF-resident a_self column. Per 128-edge
    subtile a bf16 selection matrix P_T[e,d] = (dst_local[e]==d)*p[e]
    (one DVE op) PSUM-accumulates P_T.T @ [feat|ones]: segment-softmax
    numerator and denominator fall out of one bf16 matmul chain
    (max-subtraction dropped: scores bounded ~14, exp safe in fp32).
  - h = relu(accum/denom + x@Wres^T); per-graph pooling is a one-hot
    matmul into a persistent PSUM tile; host sums the 8 partial phis.
"""
import numpy as np

import concourse.bass as bass
import concourse.tile as tile
from concourse import bacc, mybir
from concourse.bass_utils import run_bass_kernel_spmd

P = 128
D = 128
N_NODES = 10000
N_GRAPHS = 64
N_BLOCKS = 2
CORES = 8
NT = 80                 # node tiles (padded): NT*P = 10240
TPC = NT // CORES       # dst tiles owned per core
TW = 256                # bf16 table row: feat(128) | ones | a_nb | pad(126)
NEG_SLOPE = 0.01
CHUNK = 4               # subtiles per O^T broadcast chunk (<= 512 psum cols)

F32 = mybir.dt.float32
BF16 = mybir.dt.bfloat16
I16 = mybir.dt.int16

LAST_RESULTS = None     # stash for test harnesses
LAST_NC = None
LAST_IN_MAPS = None


def build_program(nt, tpc, J, n_graphs):
    """Build the SPMD Bass program (same program on all cores; per-core
    behavior comes only from input data)."""
    from contextlib import ExitStack

    nc = bacc.Bacc(
        "TRN2", target_bir_lowering=False, debug=False,
        enable_asserts=False, num_devices=1, num_swdge_queues=4,
    )
    npad = nt * P
    own = tpc * P

    # ---- DRAM I/O ----
    xt = nc.dram_tensor("xt", [nt, P, P], F32, kind="ExternalInput")
    rhs_big = nc.dram_tensor("rhs_big", [N_BLOCKS, P, 259], F32, kind="ExternalInput")
    bias_big = nc.dram_tensor("bias_big", [N_BLOCKS, P, 259], F32, kind="ExternalInput")
    gidx = nc.dram_tensor("gidx", [tpc, P, 8 * J], I16, kind="ExternalInput")
    dstlocT = nc.dram_tensor("dstlocT", [tpc, 1, J * P], BF16, kind="ExternalInput")
    dstloc = nc.dram_tensor("dstloc", [tpc, P, J], F32, kind="ExternalInput")
    gidloc = nc.dram_tensor("gidloc", [tpc, P, 1], F32, kind="ExternalInput")
    iota_d = nc.dram_tensor("iota_d", [P, P], BF16, kind="ExternalInput")
    iota_g = nc.dram_tensor("iota_g", [P, n_graphs], F32, kind="ExternalInput")
    iota_c = nc.dram_tensor("iota_c", [P, 1], F32, kind="ExternalInput")
    ones_r = nc.dram_tensor("ones_r", [1, P], BF16, kind="ExternalInput")
    ztile = nc.dram_tensor("ztile", [P, TW], BF16, kind="ExternalInput")

    h_own = nc.dram_tensor("h_own", [own, N_BLOCKS * D], F32, kind="ExternalOutput")
    phis_part = nc.dram_tensor("phis_part", [n_graphs, N_BLOCKS * D], F32,
                               kind="ExternalOutput")

    tables = [nc.dram_tensor(f"table{b}", [npad + 1, TW], BF16)
              for b in range(N_BLOCKS)]
    xress = [nc.dram_tensor(f"xres{b}", [own, D], F32) for b in range(N_BLOCKS)]

    with tile.TileContext(nc) as tc, ExitStack() as ctx:
        cpool = ctx.enter_context(tc.tile_pool(name="consts", bufs=1))
        p1 = ctx.enter_context(tc.tile_pool(name="p1", bufs=3))
        p1ps = ctx.enter_context(tc.tile_pool(name="p1ps", bufs=2, space="PSUM"))
        idxp = ctx.enter_context(tc.tile_pool(name="idxp", bufs=2))
        gp = ctx.enter_context(tc.tile_pool(name="gp", bufs=2))
        sm = ctx.enter_context(tc.tile_pool(name="sm", bufs=2))
        ptp = ctx.enter_context(tc.tile_pool(name="ptp", bufs=4))
        otp = ctx.enter_context(tc.tile_pool(name="otp", bufs=3))
        repps = ctx.enter_context(tc.tile_pool(name="repps", bufs=2, space="PSUM"))
        asps = ctx.enter_context(tc.tile_pool(name="asps", bufs=1, space="PSUM"))
        accps = ctx.enter_context(tc.tile_pool(name="accps", bufs=2, space="PSUM"))
        phips = ctx.enter_context(tc.tile_pool(name="phips", bufs=1, space="PSUM"))

        # ---- constants (host-provided; keeps gpsimd mlp-library-only) ----
        c_iota_d = cpool.tile([P, P], BF16)
        nc.sync.dma_start(c_iota_d[:], iota_d[:])
        c_iota_g = cpool.tile([P, n_graphs], F32)
        nc.sync.dma_start(c_iota_g[:], iota_g[:])
        c_iota_c = cpool.tile([P, 1], F32)
        nc.sync.dma_start(c_iota_c[:], iota_c[:])
        c_ones = cpool.tile([1, P], BF16)
        nc.sync.dma_start(c_ones[:], ones_r[:])
        c_zt = cpool.tile([P, TW], BF16)
        nc.sync.dma_start(c_zt[:], ztile[:])
        rhs_sb, bias_sb, aself_sb = [], [], []
        for b in range(N_BLOCKS):
            r = cpool.tile([P, 259], F32, tag=f"rhs{b}")
            nc.sync.dma_start(r[:], rhs_big[b])
            rhs_sb.append(r)
            bb = cpool.tile([P, 259], F32, tag=f"bias{b}")
            nc.sync.dma_start(bb[:], bias_big[b])
            bias_sb.append(bb)
            asb = cpool.tile([P, tpc], BF16, tag=f"aself{b}")
            aself_sb.append(asb)

        for b in range(N_BLOCKS):
            table, xres = tables[b], xress[b]
            # zero the whole table (incl. pad row + junk cols) first
            for k in range(0, npad + 1, P):
                rem = min(P, npad + 1 - k)
                nc.sync.dma_start(table[k:k + rem, :], c_zt[0:rem, :])

            # ---- phase 1: fused node projections ----
            # psum cols: 0:128 feat | 128 ones | 129 a_nb | 130 a_self | 131:259 xres
            for t in range(nt):
                xT = p1.tile([P, P], F32, tag="xT")
                nc.sync.dma_start(xT[:], xt[t])
                ps = p1ps.tile([P, 259], F32, tag="p1psum")
                nc.tensor.matmul(ps[:], lhsT=xT[:], rhs=rhs_sb[b][:],
                                 start=True, stop=True)
                osb = p1.tile([P, 259], F32, tag="osb")
                nc.vector.tensor_add(osb[:], ps[:], bias_sb[b][:])
                tbf = p1.tile([P, 130], BF16, tag="tbf")
                nc.scalar.copy(tbf[:], osb[:, 0:130])
                nc.sync.dma_start(table[t * P:(t + 1) * P, 0:130], tbf[:])
                if t < tpc:
                    nc.vector.tensor_copy(aself_sb[b][:, t:t + 1], osb[:, 130:131])
                    nc.sync.dma_start(xres[t * P:(t + 1) * P, :], osb[:, 131:259])

            # ---- phase 2: per-dst-tile edge aggregation ----
            php = phips.tile([n_graphs, D], F32, tag="php")
            for tt in range(tpc):
                t_gi = idxp.tile([P, 8 * J], I16, tag="tgi")
                nc.sync.dma_start(t_gi[:], gidx[tt])
                t_dlT = idxp.tile([1, J * P], BF16, tag="tdlT")
                nc.sync.dma_start(t_dlT[:], dstlocT[tt])
                t_dloc = idxp.tile([P, J], F32, tag="tdloc")
                nc.sync.dma_start(t_dloc[:], dstloc[tt])
                t_gid = idxp.tile([P, 1], F32, tag="tgid")
                nc.sync.dma_start(t_gid[:], gidloc[tt])

                G = gp.tile([P, J * TW], BF16, tag="G")
                nsplit = 4
                bounds = [round(s * J / nsplit) for s in range(nsplit + 1)]
                for s in range(nsplit):
                    j0, j1 = bounds[s], bounds[s + 1]
                    if j1 == j0:
                        continue
                    nc.gpsimd.dma_gather(
                        G[:, j0 * TW:j1 * TW].rearrange(
                            "p (j w) -> p j w", j=j1 - j0),
                        table[:],
                        t_gi[:, j0 * 8:j1 * 8],
                        (j1 - j0) * P,
                        (j1 - j0) * P,
                        TW,
                        single_packet=False,
                        queue_num=(tt * nsplit + s) % 4,
                    )

                # a_self[dst] per edge: PE row-broadcast -> onehot^T -> matmul
                ps_as = asps.tile([P, J], F32, tag="ps_as")
                for j0 in range(0, J, CHUNK):
                    cw = min(CHUNK, J - j0)
                    ps_rep = repps.tile([P, CHUNK * P], F32, tag="ps_rep")
                    nc.tensor.matmul(
                        ps_rep[:, 0:cw * P], lhsT=c_ones[:],
                        rhs=t_dlT[:, j0 * P:(j0 + cw) * P],
                        start=True, stop=True)
                    oTc = otp.tile([P, CHUNK * P], BF16, tag="oTc")
                    nc.vector.tensor_scalar(
                        out=oTc[:, 0:cw * P], in0=ps_rep[:, 0:cw * P],
                        scalar1=c_iota_c[:, 0:1], scalar2=None,
                        op0=mybir.AluOpType.is_equal)
                    for j in range(j0, j0 + cw):
                        nc.tensor.matmul(
                            ps_as[:, j:j + 1], lhsT=oTc[:, (j - j0) * P:(j - j0 + 1) * P],
                            rhs=aself_sb[b][:, tt:tt + 1],
                            start=True, stop=True)

                # batched scores: p = exp(leakyrelu(a_self + a_nb))
                t_s = sm.tile([P, J], F32, tag="ts")
                nc.vector.tensor_add(
                    t_s[:],
                    G[:].rearrange("p (j w) -> p j w", j=J)[:, :, 129],
                    ps_as[:])
                t_u = sm.tile([P, J], F32, tag="tu")
                nc.vector.scalar_tensor_tensor(
                    out=t_u[:], in0=t_s[:], scalar=NEG_SLOPE, in1=t_s[:],
                    op0=mybir.AluOpType.mult, op1=mybir.AluOpType.max)
                t_p = sm.tile([P, J], F32, tag="tp")
                nc.scalar.activation(t_p[:], t_u[:], mybir.ActivationFunctionType.Exp)

                acc = accps.tile([P, D + 1], F32, tag="acc")
                for j in range(J):
                    PT = ptp.tile([P, P], BF16, tag="PT")
                    pt_eng = nc.gpsimd if (j % 3 == 2) else nc.vector
                    pt_eng.tensor_scalar(
                        out=PT[:], in0=c_iota_d[:],
                        scalar1=t_dloc[:, j:j + 1], scalar2=t_p[:, j:j + 1],
                        op0=mybir.AluOpType.is_equal, op1=mybir.AluOpType.mult)
                    nc.tensor.matmul(acc[:], lhsT=PT[:],
                                     rhs=G[:, j * TW:j * TW + D + 1],
                                     start=(j == 0), stop=(j == J - 1))

                # h = relu(accum/denom + xres)
                dmax = sm.tile([P, 1], F32, tag="dmax")
                nc.vector.tensor_scalar_max(dmax[:], acc[:, D:D + 1], 1e-30)
                rec = sm.tile([P, 1], F32, tag="rec")
                nc.vector.reciprocal(rec[:], dmax[:])
                xr = sm.tile([P, D], F32, tag="xr")
                nc.sync.dma_start(xr[:], xres[tt * P:(tt + 1) * P, :])
                hpre = sm.tile([P, D], F32, tag="hpre")
                nc.vector.scalar_tensor_tensor(
                    out=hpre[:], in0=acc[:, 0:D], scalar=rec[:, 0:1], in1=xr[:],
                    op0=mybir.AluOpType.mult, op1=mybir.AluOpType.add)
                ht = sm.tile([P, D], F32, tag="ht")
                nc.scalar.activation(ht[:], hpre[:], mybir.ActivationFunctionType.Relu)
                nc.sync.dma_start(
                    h_own[tt * P:(tt + 1) * P, b * D:(b + 1) * D], ht[:])

                # per-graph pooling: phis += onehot(gid).T @ h
                oh = sm.tile([P, n_graphs], F32, tag="oh")
                nc.vector.tensor_scalar(
                    out=oh[:], in0=c_iota_g[:], scalar1=t_gid[:, 0:1], scalar2=None,
                    op0=mybir.AluOpType.is_equal)
                nc.tensor.matmul(php[:], lhsT=oh[:], rhs=ht[:],
                                 start=(tt == 0), stop=(tt == tpc - 1))
                if tt == tpc - 1:
                    phsb = sm.tile([n_graphs, D], F32, tag="phsb")
                    nc.vector.tensor_copy(phsb[:], php[:])
                    nc.sync.dma_start(phis_part[:, b * D:(b + 1) * D], phsb[:])

    nc.compile()
    return nc


def host_prep(x, edge_src, edge_dst, graph_ids, Wfc, bfc, wl, wr, Wres,
              cores=CORES, nt=NT, ng=N_GRAPHS):
    """Build per-core input maps + the shared J (max subtiles per dst tile)."""
    tpc = nt // cores
    npad = nt * P
    own = tpc * P
    x = np.asarray(x, dtype=np.float32)
    edge_src = np.asarray(edge_src, dtype=np.int64)
    edge_dst = np.asarray(edge_dst, dtype=np.int64)
    graph_ids = np.asarray(graph_ids)
    Wfc = np.asarray(Wfc, dtype=np.float32)
    bfc = np.asarray(bfc, dtype=np.float32)
    wl = np.asarray(wl, dtype=np.float32)
    wr = np.asarray(wr, dtype=np.float32)
    Wres = np.asarray(Wres, dtype=np.float32)

    n = x.shape[0]
    x_pad = np.zeros((npad, D), dtype=np.float32)
    x_pad[:n] = x

    # fused weights: cols 0:128 Wfc^T | 128 zeros(->ones via bias) | 129 v_r |
    # 130 v_l | 131:259 Wres^T
    rhs_big = np.zeros((N_BLOCKS, P, 259), dtype=np.float32)
    bias_big = np.zeros((N_BLOCKS, P, 259), dtype=np.float32)
    for b in range(N_BLOCKS):
        rhs_big[b, :, 0:D] = Wfc[b].T
        rhs_big[b, :, 129] = Wfc[b].T @ wr[b]
        rhs_big[b, :, 130] = Wfc[b].T @ wl[b]
        rhs_big[b, :, 131:259] = Wres[b].T
        row = np.zeros(259, dtype=np.float32)
        row[0:D] = bfc[b]
        row[128] = 1.0
        row[129] = float(bfc[b] @ wr[b])
        row[130] = float(bfc[b] @ wl[b])
        bias_big[b, :, :] = row[None, :]

    import ml_dtypes
    consts = {
        "iota_d": np.tile(np.arange(P).astype(ml_dtypes.bfloat16)[None, :], (P, 1)),
        "iota_g": np.tile(np.arange(ng, dtype=np.float32)[None, :], (P, 1)),
        "iota_c": np.arange(P, dtype=np.float32)[:, None],
        "ones_r": np.ones((1, P), dtype=ml_dtypes.bfloat16),
        "ztile": np.zeros((P, TW), dtype=ml_dtypes.bfloat16),
    }

    # bucket edges by global dst tile
    order = np.argsort(edge_dst, kind="stable")
    sdst = edge_dst[order]
    ssrc = edge_src[order]
    tile_of = sdst // P
    counts = np.bincount(tile_of, minlength=nt)
    J = int(np.ceil(counts.max() / P))
    starts = np.zeros(nt + 1, dtype=np.int64)
    np.cumsum(counts, out=starts[1:])

    in_maps = []
    for c in range(cores):
        lo, hi = c * own, (c + 1) * own
        perm = np.concatenate([
            np.arange(lo, hi), np.arange(0, lo), np.arange(hi, npad)])
        pos = np.empty(npad, dtype=np.int64)
        pos[perm] = np.arange(npad)

        xtc = np.ascontiguousarray(
            x_pad[perm].reshape(nt, P, D).transpose(0, 2, 1))

        gidx_a = np.zeros((tpc, P, 8 * J), dtype=np.int16)
        dlT_a = np.full((tpc, 1, J * P), -1.0, dtype=ml_dtypes.bfloat16)
        dloc_a = np.full((tpc, P, J), -1.0, dtype=np.float32)
        for tt in range(tpc):
            g = c * tpc + tt
            s, e = starts[g], starts[g + 1]
            cnt = e - s
            flat_src = np.full(J * P, npad, dtype=np.int16)
            flat_src[:cnt] = pos[ssrc[s:e]].astype(np.int16)
            flat_dl = np.full(J * P, -1.0, dtype=np.float32)
            flat_dl[:cnt] = (sdst[s:e] - g * P).astype(np.float32)
            # wrapped-16 idx layout, replicated for the 8 gpsimd cores
            base = flat_src.reshape(8 * J, 16).T        # [16, 8J]
            gidx_a[tt] = np.tile(base, (8, 1))
            dlT_a[tt, 0] = flat_dl.astype(ml_dtypes.bfloat16)
            dloc_a[tt] = flat_dl.reshape(J, P).T

        gid_a = np.full((tpc, P, 1), -1.0, dtype=np.float32)
        own_nodes = np.arange(lo, hi)
        valid = own_nodes < n
        gv = np.full(own, -1.0, dtype=np.float32)
        gv[valid] = graph_ids[own_nodes[valid]].astype(np.float32)
        gid_a[:, :, 0] = gv.reshape(tpc, P)

        m = {"xt": xtc, "rhs_big": rhs_big, "bias_big": bias_big,
             "gidx": gidx_a, "dstlocT": dlT_a, "dstloc": dloc_a,
             "gidloc": gid_a}
        m.update(consts)
        in_maps.append(m)
    return in_maps, J


def kernel(x, edge_src, edge_dst, graph_ids, Wfc, bfc, wl, wr, Wres):
    global LAST_RESULTS, LAST_NC, LAST_IN_MAPS
    in_maps, J = host_prep(x, edge_src, edge_dst, graph_ids, Wfc, bfc, wl,
                           wr, Wres)
    nc = build_program(NT, TPC, J, N_GRAPHS)
    res = run_bass_kernel_spmd(nc, in_maps, list(range(CORES)))
    LAST_RESULTS = res
    LAST_NC, LAST_IN_MAPS = nc, in_maps

    n = np.asarray(x).shape[0]
    h = np.concatenate([r["h_own"] for r in res.results], axis=0)[:n]
    phis = np.sum([r["phis_part"] for r in res.results], axis=0,
                  dtype=np.float64).astype(np.float32)
    return h.astype(np.float32), phis
